# revision 37
# baseline (speedup 1.0000x reference)
"""Trainium2 Bass kernel for nn_CustomLinear (block-sparse QKV projection).

Given x (8, 4096, 130), per-head 64x64 blocks M_q/M_k (4,64,64), M_v
(8,64,64) and scalar biases B_q/B_k (8,1,1), produces q, k, v each of shape
(8, 4096, 1040) = (B, N, H*E).  Per token row of 1040 floats, only a few
column blocks are nonzero:

  q: head h<4 : cols 130h+65..128  = M_q[h] @ x2,   col 130h+129 = s_last*bq[h]
     head h>=4: col  130h+65       = s_last*bq[h]
  k: head h<4 : cols 130h+65..128  = M_k[h] @ x1,   col 130h+129 = s_last*bk[h]
     head h>=4: col  130h+65       = s_mid*bk[h]
  v: all heads: cols 130h+65..128  = M_v[h] @ x1
  (x1 = x cols 0:64, x2 = x cols 65:129, s_mid = x col 64, s_last = x col 129)

Sharding: pure data parallelism, one batch row per NeuronCore (8 cores),
the tiny weights replicated.

The device computes ONLY the 1024 matmul-block output columns per token
(the 16 bias columns are rank-1 scalar products the host forms directly
from x's s_mid/s_last columns), in fp16 (tolerance is 2e-2; fp16
end-to-end is ~5e-4).  Contraction is exactly K=128 = [x1; x2], so each
128-token tile is ONE stationary ldweights + two 512-col fp16 matmuls
filling two full PSUM banks, one (128, 1024) f32->f16 PSUM->SBUF copy
(alternating DVE/Act), and per 512-token macro one output DMA of 128
contiguous 8 KB descriptors (partition-major DRAM layout, un-permuted on
the host).  The two HWDGE rings alternate macros; HWDGE descriptor
dispatch (~18 ns/desc) and the 16 DMA engines (~22.5 GB/s each) both sit
just under the ~24 us HBM write floor for the ~8.4 MB/core of output.

Host side packs x/weights into fp16 matmul operands and scatters the
compact device output into the structurally-zero full (8, 4096, 1040)
tensors (pure layout + one tiny rank-1 bias product).
"""

import numpy as np
from contextlib import ExitStack

import concourse.bass as bass
import concourse.bacc as bacc
import concourse.mybir as mybir
import concourse.tile as tile
from concourse.bass_utils import run_bass_kernel_spmd

F32 = mybir.dt.float32
F16 = mybir.dt.float16

B = 8            # batches == cores
N = 4096         # tokens per core
D = 64
H = 8            # heads
P = 4            # pair heads
E = 130
HE = H * E       # 1040
KC = 128         # contraction rows: x1 (64) + x2 (64)
OC = 1024        # compact output cols: k 4*64 | v 8*64 | q 4*64
SUB = 128        # tokens per matmul
# input block sizes (tokens): a small first block so the first matmul can
# start ASAP, bigger ones after (SWDGE transfers interleave round-robin, so
# block k only has to beat the PE to token XBLK[k])
XBLK = [512, 1536, 2048]
NCHUNK = N // SUB            # 32 token chunks in the partition-major output
# Macro schedule (tok0, nsub): small macros first so the output DMA stream
# starts early, then growing macros (up to 16 KB per-partition descriptors)
# to maximize DMA-engine utilization, and a small last macro so the drain
# tail is short.  Every macro owns a DEDICATED stage buffer (64 KB per
# partition total) so compute never stalls on a stage WAR against an
# in-flight DMA — the backlog simply builds in SBUF and the two HWDGE
# rings drain it at the full DMA-engine rate.
SCHED = [(0, 2), (2 * SUB, 2)] + [
    (t, 4) for t in range(4 * SUB, N, 4 * SUB)
]
assert sum(ns for _, ns in SCHED) * SUB == N
assert all(t == sum(n for _, n in SCHED[:i]) * SUB for i, (t, _) in enumerate(SCHED))
NWARM = 5        # PE warm-up matmuls during the input-DMA wait: enough to
                 # keep the PE busy (and its DVFS ramping) until the first
                 # x block lands at ~9.5 us, without overshooting past it

_CACHE = {}


def _build():
    # Bacc (not raw Bass): its compile() legalizes the TRN2 one-sync-wait-
    # per-instruction constraint (move_matmul_waits_to_ldweights +
    # generate_event_semaphores), which walrus codegen hard-requires.
    nc = bacc.Bacc("TRN2", target_bir_lowering=False, debug=False)
    # xp rows: x1 rows 0:64, x2 rows 64:128
    xp = nc.dram_tensor("xp", [KC, N], F16, kind="ExternalInput").ap()
    wp = nc.dram_tensor("wp", [KC, OC], F16, kind="ExternalInput").ap()
    # partition-major compact output: o[p, c, :] = token c*128+p
    o = nc.dram_tensor("o", [SUB, NCHUNK, OC], F16, kind="ExternalOutput").ap()

    with tile.TileContext(nc) as tc, ExitStack() as ctx:
        wpool = ctx.enter_context(tc.tile_pool(name="wpool", bufs=1))
        xpool = ctx.enter_context(tc.tile_pool(name="xpool", bufs=1))
        opool = ctx.enter_context(tc.tile_pool(name="opool", bufs=1))
        pspool = ctx.enter_context(tc.tile_pool(name="pspool", bufs=4, space="PSUM"))

        # weights arrive early on the (fast, otherwise-idle-at-start) sync
        # HWDGE ring; ALL x blocks prefetch on the SWDGE ring so the output
        # stream owns the sync ring afterwards
        wsb = wpool.tile([KC, OC], F16, name="wsb")
        nc.sync.dma_start(wsb[:], wp[:])

        # PE warm-up memset first on the gpsimd queue (it is ~100 ns; the
        # input descriptor generations behind it are what the warm matmuls
        # are hiding)
        warm_sb = wpool.tile([SUB, 640], F16, name="warm_sb")
        nc.gpsimd.memset(warm_sb[:], 0.0)
        warm_ps = pspool.tile([SUB, 512], F32, tag="warm", name="warm", bufs=1)
        for _ in range(NWARM):
            nc.tensor.matmul(warm_ps[:], warm_sb[:, 0:SUB],
                             warm_sb[:, SUB:640], start=True, stop=True)

        xts = []   # (start_token, end_token, tile)
        tok = 0
        for blk, w in enumerate(XBLK):
            xt = xpool.tile([KC, w], F16, name=f"xt{blk}")
            nc.gpsimd.dma_start(xt[:], xp[:, tok:tok + w])
            xts.append((tok, tok + w, xt))
            tok += w
        assert tok == N

        stage = [
            opool.tile([SUB, nsub * OC], F16, name=f"st{i}")
            for i, (_, nsub) in enumerate(SCHED)
        ]

        cp = 0  # copy-engine round-robin
        for m, (tok0, nsub) in enumerate(SCHED):
            st = stage[m]
            for s in range(nsub):
                tok = tok0 + s * SUB
                blk0, _, xt = next(b for b in xts if b[0] <= tok < b[1])
                lo = tok - blk0
                off = s * OC
                # one shared stationary (the x tile) per sub-tile; two
                # 512-col fp16 matmuls (matmul free size is capped at one
                # 2 KB PSUM bank) fill a 2-bank PSUM tile exactly
                ps = pspool.tile([SUB, OC], F32, tag="ps", name="ps", bufs=3)
                nc.tensor.matmul(ps[:, 0:512], xt[:, lo:lo + SUB],
                                 wsb[:, 0:512], start=True, stop=True)
                nc.tensor.matmul(ps[:, 512:1024], xt[:, lo:lo + SUB],
                                 wsb[:, 512:1024], start=True, stop=True)
                # f32 PSUM -> f16 stage copy, alternating DVE / Act
                eng = nc.vector.tensor_copy if cp % 2 == 0 else nc.scalar.copy
                eng(st[:, off:off + OC], ps[:])
                cp += 1

            # Output DMAs ride the sync ring: a single queue with standing
            # backlog keeps ~19 descriptors in flight (engine-limited), and
            # the Act engine runs nothing but copies (a trigger on Act would
            # head-of-line-block its copy stream behind the DMA's wait).
            # Three mid-kernel macros go out on the gpsimd SWDGE queue — idle
            # after the input prefetch — as a second concurrent stream to
            # fill the sync queue's instruction-boundary idle.
            dst = o[:, tok0 // SUB:tok0 // SUB + nsub, :]
            src = st[:].rearrange("p (s e) -> p s e", e=OC)
            eng = nc.gpsimd if m in (3, 5, 7) else nc.sync
            eng.dma_start(dst, src)
    nc.compile()
    return nc


def _pack_weights(M_q, M_k, M_v):
    w = np.zeros((KC, OC), np.float32)
    for h in range(P):                       # K blocks: cols 0:256 <- x1
        w[0:64, h * 64:(h + 1) * 64] = M_k[h].T
    for h in range(H):                       # V blocks: cols 256:768 <- x1
        w[0:64, 256 + h * 64:256 + (h + 1) * 64] = M_v[h].T
    for h in range(P):                       # Q blocks: cols 768:1024 <- x2
        w[64:128, 768 + h * 64:768 + (h + 1) * 64] = M_q[h].T
    return w


def _prep_inputs(inputs):
    x = np.asarray(inputs["x"], np.float32)
    M_q = np.asarray(inputs["M_q"], np.float32)
    M_k = np.asarray(inputs["M_k"], np.float32)
    M_v = np.asarray(inputs["M_v"], np.float32)
    wp = _pack_weights(M_q, M_k, M_v).astype(np.float16)

    in_maps = []
    for b in range(B):
        xt = x[b].T  # (130, 4096) view
        xpk = np.empty((KC, N), np.float16)
        xpk[0:64] = xt[0:64]       # x1 rows
        xpk[64:128] = xt[65:129]   # x2 rows
        in_maps.append({"xp": xpk, "wp": wp})
    return in_maps


def _unpack_outputs(inputs, res):
    x = np.asarray(inputs["x"], np.float32)
    B_q = np.asarray(inputs["B_q"], np.float32)[:, 0, 0]
    B_k = np.asarray(inputs["B_k"], np.float32)[:, 0, 0]
    s_mid = x[:, :, 64]
    s_last = x[:, :, 129]

    # (B, 128, 32, 1024) partition-major -> token-major (B, N, 1024)
    oc = np.stack([np.asarray(res.results[b]["o"]) for b in range(B)])
    oc = oc.transpose(0, 2, 1, 3).reshape(B, N, OC)
    kc = oc[:, :, 0:256]
    vc = oc[:, :, 256:768]
    qc = oc[:, :, 768:1024]

    def qk_full(c, pair_bias, high_bias):
        f = np.zeros((B, N, H, E), np.float32)
        f[:, :, :P, 65:129] = c.reshape(B, N, P, 64)
        f[:, :, :P, 129] = pair_bias
        f[:, :, P:, 65] = high_bias
        return f.reshape(B, N, HE)

    q = qk_full(qc, s_last[..., None] * B_q[:P], s_last[..., None] * B_q[P:])
    k = qk_full(kc, s_last[..., None] * B_k[:P], s_mid[..., None] * B_k[P:])
    v_full = np.zeros((B, N, H, E), np.float32)
    v_full[:, :, :, 65:129] = vc.reshape(B, N, H, 64)
    return q, k, v_full.reshape(B, N, HE)


def _run(inputs, trace=False):
    if "nc" not in _CACHE:
        _CACHE["nc"] = _build()
    nc = _CACHE["nc"]
    in_maps = _prep_inputs(inputs)
    res = run_bass_kernel_spmd(nc, in_maps, core_ids=list(range(B)), trace=trace)
    return _unpack_outputs(inputs, res), res


def kernel(**inputs):
    outs, _ = _run(inputs, trace=False)
    return outs


# revision 39
# speedup vs baseline: 1.1559x; 1.1559x over previous
"""Trainium2 Bass kernel for nn_CustomLinear (block-sparse QKV projection).

Given x (8, 4096, 130), per-head 64x64 blocks M_q/M_k (4,64,64), M_v
(8,64,64) and scalar biases B_q/B_k (8,1,1), produces q, k, v each of shape
(8, 4096, 1040) = (B, N, H*E).  Per token row of 1040 floats, only a few
column blocks are nonzero:

  q: head h<4 : cols 130h+65..128  = M_q[h] @ x2,   col 130h+129 = s_last*bq[h]
     head h>=4: col  130h+65       = s_last*bq[h]
  k: head h<4 : cols 130h+65..128  = M_k[h] @ x1,   col 130h+129 = s_last*bk[h]
     head h>=4: col  130h+65       = s_mid*bk[h]
  v: all heads: cols 130h+65..128  = M_v[h] @ x1
  (x1 = x cols 0:64, x2 = x cols 65:129, s_mid = x col 64, s_last = x col 129)

Sharding: pure data parallelism, one batch row per NeuronCore (8 cores),
the tiny weights replicated.

The device computes ONLY the 1024 matmul-block output columns per token
(the 16 bias columns are rank-1 scalar products the host forms directly
from x's s_mid/s_last columns), in fp16 (tolerance is 2e-2; fp16
end-to-end is ~5e-4).  Contraction is exactly K=128 = [x1; x2], so each
128-token tile is ONE stationary ldweights + two 512-col fp16 matmuls
filling two full PSUM banks, one (128, 1024) f32->f16 PSUM->SBUF copy
(alternating DVE/Act), and per 512-token macro one output DMA of 128
contiguous 8 KB descriptors (partition-major DRAM layout, un-permuted on
the host).  The two HWDGE rings alternate macros; HWDGE descriptor
dispatch (~18 ns/desc) and the 16 DMA engines (~22.5 GB/s each) both sit
just under the ~24 us HBM write floor for the ~8.4 MB/core of output.

Host side packs x/weights into fp16 matmul operands and scatters the
compact device output into the structurally-zero full (8, 4096, 1040)
tensors (pure layout + one tiny rank-1 bias product).
"""

import numpy as np
from contextlib import ExitStack

import concourse.bass as bass
import concourse.bacc as bacc
import concourse.mybir as mybir
import concourse.tile as tile
from concourse.bass_utils import run_bass_kernel_spmd

F32 = mybir.dt.float32
F16 = mybir.dt.float16

B = 8            # batches == cores
N = 4096         # tokens per core
D = 64
H = 8            # heads
P = 4            # pair heads
E = 130
HE = H * E       # 1040
KC = 128         # contraction rows: x1 (64) + x2 (64)
OC = 1024        # compact output cols: k 4*64 | v 8*64 | q 4*64
SUB = 128        # tokens per matmul
# input block sizes (tokens): a small first block so the first matmul can
# start ASAP, bigger ones after (SWDGE transfers interleave round-robin, so
# block k only has to beat the PE to token XBLK[k])
XBLK = [512, 1536, 2048]
NCHUNK = N // SUB            # 32 token chunks in the partition-major output
# Macro schedule (tok0, nsub): small macros first so the output DMA stream
# starts early, then growing macros (up to 16 KB per-partition descriptors)
# to maximize DMA-engine utilization, and a small last macro so the drain
# tail is short.  Every macro owns a DEDICATED stage buffer (64 KB per
# partition total) so compute never stalls on a stage WAR against an
# in-flight DMA — the backlog simply builds in SBUF and the two HWDGE
# rings drain it at the full DMA-engine rate.
SCHED = [(0, 2), (2 * SUB, 2)] + [
    (t, 4) for t in range(4 * SUB, 28 * SUB, 4 * SUB)
] + [(28 * SUB, 2), (30 * SUB, 2)]
assert sum(ns for _, ns in SCHED) * SUB == N
assert all(t == sum(n for _, n in SCHED[:i]) * SUB for i, (t, _) in enumerate(SCHED))
NWARM = 5        # PE warm-up matmuls during the input-DMA wait: enough to
                 # keep the PE busy (and its DVFS ramping) until the first
                 # x block lands at ~9.5 us, without overshooting past it

_CACHE = {}


def _build():
    # Bacc (not raw Bass): its compile() legalizes the TRN2 one-sync-wait-
    # per-instruction constraint (move_matmul_waits_to_ldweights +
    # generate_event_semaphores), which walrus codegen hard-requires.
    nc = bacc.Bacc("TRN2", target_bir_lowering=False, debug=False)
    # xp rows: x1 rows 0:64, x2 rows 64:128
    xp = nc.dram_tensor("xp", [KC, N], F16, kind="ExternalInput").ap()
    wp = nc.dram_tensor("wp", [KC, OC], F16, kind="ExternalInput").ap()
    # partition-major compact output: o[p, c, :] = token c*128+p
    o = nc.dram_tensor("o", [SUB, NCHUNK, OC], F16, kind="ExternalOutput").ap()

    with tile.TileContext(nc) as tc, ExitStack() as ctx:
        wpool = ctx.enter_context(tc.tile_pool(name="wpool", bufs=1))
        xpool = ctx.enter_context(tc.tile_pool(name="xpool", bufs=1))
        opool = ctx.enter_context(tc.tile_pool(name="opool", bufs=1))
        pspool = ctx.enter_context(tc.tile_pool(name="pspool", bufs=4, space="PSUM"))

        # weights arrive early on the (fast, otherwise-idle-at-start) sync
        # HWDGE ring; ALL x blocks prefetch on the SWDGE ring so the output
        # stream owns the sync ring afterwards
        wsb = wpool.tile([KC, OC], F16, name="wsb")
        nc.sync.dma_start(wsb[:], wp[:])

        # PE warm-up memset first on the gpsimd queue (it is ~100 ns; the
        # input descriptor generations behind it are what the warm matmuls
        # are hiding)
        warm_sb = wpool.tile([SUB, 640], F16, name="warm_sb")
        nc.gpsimd.memset(warm_sb[:], 0.0)
        for _ in range(NWARM):
            # warm matmuls rotate through the regular 4-deep PSUM tiles so
            # all 8 banks serve the real pipeline
            warm_ps = pspool.tile([SUB, OC], F32, tag="ps", name="ps", bufs=4)
            nc.tensor.matmul(warm_ps[:, 0:512], warm_sb[:, 0:SUB],
                             warm_sb[:, SUB:640], start=True, stop=True)

        xts = []   # (start_token, end_token, tile)
        tok = 0
        for blk, w in enumerate(XBLK):
            xt = xpool.tile([KC, w], F16, name=f"xt{blk}")
            nc.gpsimd.dma_start(xt[:], xp[:, tok:tok + w])
            xts.append((tok, tok + w, xt))
            tok += w
        assert tok == N

        stage = [
            opool.tile([SUB, nsub * OC], F16, name=f"st{i}")
            for i, (_, nsub) in enumerate(SCHED)
        ]

        cp = 0  # copy-engine round-robin
        for m, (tok0, nsub) in enumerate(SCHED):
            st = stage[m]
            for s in range(nsub):
                tok = tok0 + s * SUB
                blk0, _, xt = next(b for b in xts if b[0] <= tok < b[1])
                lo = tok - blk0
                off = s * OC
                # one shared stationary (the x tile) per sub-tile; two
                # 512-col fp16 matmuls (matmul free size is capped at one
                # 2 KB PSUM bank) fill a 2-bank PSUM tile exactly
                ps = pspool.tile([SUB, OC], F32, tag="ps", name="ps", bufs=4)
                nc.tensor.matmul(ps[:, 0:512], xt[:, lo:lo + SUB],
                                 wsb[:, 0:512], start=True, stop=True)
                nc.tensor.matmul(ps[:, 512:1024], xt[:, lo:lo + SUB],
                                 wsb[:, 512:1024], start=True, stop=True)
                # f32 PSUM -> f16 stage copy, alternating DVE / Act
                eng = nc.vector.tensor_copy if cp % 2 == 0 else nc.scalar.copy
                eng(st[:, off:off + OC], ps[:])
                cp += 1

            # Output DMAs ride the sync ring: a single queue with standing
            # backlog keeps ~19 descriptors in flight (engine-limited), and
            # the Act engine runs nothing but copies (a trigger on Act would
            # head-of-line-block its copy stream behind the DMA's wait).
            # Two mid-kernel macros go out on the gpsimd SWDGE queue — idle
            # after the input prefetch — as a second concurrent stream to
            # fill the sync queue's instruction-boundary idle.
            dst = o[:, tok0 // SUB:tok0 // SUB + nsub, :]
            src = st[:].rearrange("p (s e) -> p s e", e=OC)
            eng = nc.gpsimd if m in (4, 6) else nc.sync
            eng.dma_start(dst, src)
    nc.compile()
    return nc


def _pack_weights(M_q, M_k, M_v):
    w = np.zeros((KC, OC), np.float32)
    for h in range(P):                       # K blocks: cols 0:256 <- x1
        w[0:64, h * 64:(h + 1) * 64] = M_k[h].T
    for h in range(H):                       # V blocks: cols 256:768 <- x1
        w[0:64, 256 + h * 64:256 + (h + 1) * 64] = M_v[h].T
    for h in range(P):                       # Q blocks: cols 768:1024 <- x2
        w[64:128, 768 + h * 64:768 + (h + 1) * 64] = M_q[h].T
    return w


def _prep_inputs(inputs):
    x = np.asarray(inputs["x"], np.float32)
    M_q = np.asarray(inputs["M_q"], np.float32)
    M_k = np.asarray(inputs["M_k"], np.float32)
    M_v = np.asarray(inputs["M_v"], np.float32)
    wp = _pack_weights(M_q, M_k, M_v).astype(np.float16)

    in_maps = []
    for b in range(B):
        xt = x[b].T  # (130, 4096) view
        xpk = np.empty((KC, N), np.float16)
        xpk[0:64] = xt[0:64]       # x1 rows
        xpk[64:128] = xt[65:129]   # x2 rows
        in_maps.append({"xp": xpk, "wp": wp})
    return in_maps


def _unpack_outputs(inputs, res):
    x = np.asarray(inputs["x"], np.float32)
    B_q = np.asarray(inputs["B_q"], np.float32)[:, 0, 0]
    B_k = np.asarray(inputs["B_k"], np.float32)[:, 0, 0]
    s_mid = x[:, :, 64]
    s_last = x[:, :, 129]

    # (B, 128, 32, 1024) partition-major -> token-major (B, N, 1024)
    oc = np.stack([np.asarray(res.results[b]["o"]) for b in range(B)])
    oc = oc.transpose(0, 2, 1, 3).reshape(B, N, OC)
    kc = oc[:, :, 0:256]
    vc = oc[:, :, 256:768]
    qc = oc[:, :, 768:1024]

    def qk_full(c, pair_bias, high_bias):
        f = np.zeros((B, N, H, E), np.float32)
        f[:, :, :P, 65:129] = c.reshape(B, N, P, 64)
        f[:, :, :P, 129] = pair_bias
        f[:, :, P:, 65] = high_bias
        return f.reshape(B, N, HE)

    q = qk_full(qc, s_last[..., None] * B_q[:P], s_last[..., None] * B_q[P:])
    k = qk_full(kc, s_last[..., None] * B_k[:P], s_mid[..., None] * B_k[P:])
    v_full = np.zeros((B, N, H, E), np.float32)
    v_full[:, :, :, 65:129] = vc.reshape(B, N, H, 64)
    return q, k, v_full.reshape(B, N, HE)


def _run(inputs, trace=False):
    if "nc" not in _CACHE:
        _CACHE["nc"] = _build()
    nc = _CACHE["nc"]
    in_maps = _prep_inputs(inputs)
    res = run_bass_kernel_spmd(nc, in_maps, core_ids=list(range(B)), trace=trace)
    return _unpack_outputs(inputs, res), res


def kernel(**inputs):
    outs, _ = _run(inputs, trace=False)
    return outs


# revision 40
# speedup vs baseline: 1.2395x; 1.0723x over previous
"""Trainium2 Bass kernel for nn_CustomLinear (block-sparse QKV projection).

Given x (8, 4096, 130), per-head 64x64 blocks M_q/M_k (4,64,64), M_v
(8,64,64) and scalar biases B_q/B_k (8,1,1), produces q, k, v each of shape
(8, 4096, 1040) = (B, N, H*E).  Per token row of 1040 floats, only a few
column blocks are nonzero:

  q: head h<4 : cols 130h+65..128  = M_q[h] @ x2,   col 130h+129 = s_last*bq[h]
     head h>=4: col  130h+65       = s_last*bq[h]
  k: head h<4 : cols 130h+65..128  = M_k[h] @ x1,   col 130h+129 = s_last*bk[h]
     head h>=4: col  130h+65       = s_mid*bk[h]
  v: all heads: cols 130h+65..128  = M_v[h] @ x1
  (x1 = x cols 0:64, x2 = x cols 65:129, s_mid = x col 64, s_last = x col 129)

Sharding: pure data parallelism, one batch row per NeuronCore (8 cores),
the tiny weights replicated.

The device computes ONLY the 1024 matmul-block output columns per token
(the 16 bias columns are rank-1 scalar products the host forms directly
from x's s_mid/s_last columns), in fp16 (tolerance is 2e-2; fp16
end-to-end is ~5e-4).  Contraction is exactly K=128 = [x1; x2], so each
128-token tile is ONE stationary ldweights + two 512-col fp16 matmuls
filling two full PSUM banks, one (128, 1024) f32->f16 PSUM->SBUF copy
(alternating DVE/Act), and per 512-token macro one output DMA of 128
contiguous 8 KB descriptors (partition-major DRAM layout, un-permuted on
the host).  The two HWDGE rings alternate macros; HWDGE descriptor
dispatch (~18 ns/desc) and the 16 DMA engines (~22.5 GB/s each) both sit
just under the ~24 us HBM write floor for the ~8.4 MB/core of output.

Host side packs x/weights into fp16 matmul operands and scatters the
compact device output into the structurally-zero full (8, 4096, 1040)
tensors (pure layout + one tiny rank-1 bias product).
"""

import numpy as np
from contextlib import ExitStack

import concourse.bass as bass
import concourse.bacc as bacc
import concourse.mybir as mybir
import concourse.tile as tile
from concourse.bass_utils import run_bass_kernel_spmd

F32 = mybir.dt.float32
F16 = mybir.dt.float16

B = 8            # batches == cores
N = 4096         # tokens per core
D = 64
H = 8            # heads
P = 4            # pair heads
E = 130
HE = H * E       # 1040
KC = 128         # contraction rows: x1 (64) + x2 (64)
OC = 1024        # compact output cols: k 4*64 | v 8*64 | q 4*64
SUB = 128        # tokens per matmul
# input block sizes (tokens): a small first block so the first matmul can
# start ASAP, bigger ones after (SWDGE transfers interleave round-robin, so
# block k only has to beat the PE to token XBLK[k])
XBLK = [512, 1536, 2048]
NCHUNK = N // SUB            # 32 token chunks in the partition-major output
# Macro schedule (tok0, nsub): small macros first so the output DMA stream
# starts early, then growing macros (up to 16 KB per-partition descriptors)
# to maximize DMA-engine utilization, and a small last macro so the drain
# tail is short.  Every macro owns a DEDICATED stage buffer (64 KB per
# partition total) so compute never stalls on a stage WAR against an
# in-flight DMA — the backlog simply builds in SBUF and the two HWDGE
# rings drain it at the full DMA-engine rate.
SCHED = [(0, 2), (2 * SUB, 2)] + [
    (t, 4) for t in range(4 * SUB, N, 4 * SUB)
]
assert sum(ns for _, ns in SCHED) * SUB == N
assert all(t == sum(n for _, n in SCHED[:i]) * SUB for i, (t, _) in enumerate(SCHED))
NWARM = 5        # PE warm-up matmuls during the input-DMA wait: enough to
                 # keep the PE busy (and its DVFS ramping) until the first
                 # x block lands at ~9.5 us, without overshooting past it

_CACHE = {}


def _build():
    # Bacc (not raw Bass): its compile() legalizes the TRN2 one-sync-wait-
    # per-instruction constraint (move_matmul_waits_to_ldweights +
    # generate_event_semaphores), which walrus codegen hard-requires.
    nc = bacc.Bacc("TRN2", target_bir_lowering=False, debug=False)
    # xp rows: x1 rows 0:64, x2 rows 64:128
    xp = nc.dram_tensor("xp", [KC, N], F16, kind="ExternalInput").ap()
    wp = nc.dram_tensor("wp", [KC, OC], F16, kind="ExternalInput").ap()
    # partition-major compact output: o[p, c, :] = token c*128+p
    o = nc.dram_tensor("o", [SUB, NCHUNK, OC], F16, kind="ExternalOutput").ap()

    with tile.TileContext(nc) as tc, ExitStack() as ctx:
        wpool = ctx.enter_context(tc.tile_pool(name="wpool", bufs=1))
        xpool = ctx.enter_context(tc.tile_pool(name="xpool", bufs=1))
        opool = ctx.enter_context(tc.tile_pool(name="opool", bufs=1))
        pspool = ctx.enter_context(tc.tile_pool(name="pspool", bufs=4, space="PSUM"))

        # weights arrive early on the (fast, otherwise-idle-at-start) sync
        # HWDGE ring; ALL x blocks prefetch on the SWDGE ring so the output
        # stream owns the sync ring afterwards
        wsb = wpool.tile([KC, OC], F16, name="wsb")
        nc.sync.dma_start(wsb[:], wp[:])

        # PE warm-up memset first on the gpsimd queue (it is ~100 ns; the
        # input descriptor generations behind it are what the warm matmuls
        # are hiding)
        warm_sb = wpool.tile([SUB, 640], F16, name="warm_sb")
        nc.gpsimd.memset(warm_sb[:], 0.0)
        warm_ps = pspool.tile([SUB, 512], F32, tag="warm", name="warm", bufs=1)
        for _ in range(NWARM):
            nc.tensor.matmul(warm_ps[:], warm_sb[:, 0:SUB],
                             warm_sb[:, SUB:640], start=True, stop=True)

        xts = []   # (start_token, end_token, tile)
        tok = 0
        for blk, w in enumerate(XBLK):
            xt = xpool.tile([KC, w], F16, name=f"xt{blk}")
            nc.gpsimd.dma_start(xt[:], xp[:, tok:tok + w])
            xts.append((tok, tok + w, xt))
            tok += w
        assert tok == N

        stage = [
            opool.tile([SUB, nsub * OC], F16, name=f"st{i}")
            for i, (_, nsub) in enumerate(SCHED)
        ]

        cp = 0  # copy-engine round-robin
        for m, (tok0, nsub) in enumerate(SCHED):
            st = stage[m]
            for s in range(nsub):
                tok = tok0 + s * SUB
                blk0, _, xt = next(b for b in xts if b[0] <= tok < b[1])
                lo = tok - blk0
                off = s * OC
                # one shared stationary (the x tile) per sub-tile; two
                # 512-col fp16 matmuls (matmul free size is capped at one
                # 2 KB PSUM bank) fill a 2-bank PSUM tile exactly
                ps = pspool.tile([SUB, OC], F32, tag="ps", name="ps", bufs=3)
                nc.tensor.matmul(ps[:, 0:512], xt[:, lo:lo + SUB],
                                 wsb[:, 0:512], start=True, stop=True)
                nc.tensor.matmul(ps[:, 512:1024], xt[:, lo:lo + SUB],
                                 wsb[:, 512:1024], start=True, stop=True)
                # f32 PSUM -> f16 stage copy, alternating DVE / Act
                eng = nc.vector.tensor_copy if cp % 2 == 0 else nc.scalar.copy
                eng(st[:, off:off + OC], ps[:])
                cp += 1

            # Output DMAs ride the sync ring: a single queue with standing
            # backlog keeps ~19 descriptors in flight (engine-limited), and
            # the Act engine runs nothing but copies (a trigger on Act would
            # head-of-line-block its copy stream behind the DMA's wait).
            # Two mid-kernel macros go out on the gpsimd SWDGE queue — idle
            # after the input prefetch — as a second concurrent stream to
            # fill the sync queue's instruction-boundary idle.
            dst = o[:, tok0 // SUB:tok0 // SUB + nsub, :]
            src = st[:].rearrange("p (s e) -> p s e", e=OC)
            eng = nc.gpsimd if m in (4, 6) else nc.sync
            eng.dma_start(dst, src)
    nc.compile()
    return nc


def _pack_weights(M_q, M_k, M_v):
    w = np.zeros((KC, OC), np.float32)
    for h in range(P):                       # K blocks: cols 0:256 <- x1
        w[0:64, h * 64:(h + 1) * 64] = M_k[h].T
    for h in range(H):                       # V blocks: cols 256:768 <- x1
        w[0:64, 256 + h * 64:256 + (h + 1) * 64] = M_v[h].T
    for h in range(P):                       # Q blocks: cols 768:1024 <- x2
        w[64:128, 768 + h * 64:768 + (h + 1) * 64] = M_q[h].T
    return w


def _prep_inputs(inputs):
    x = np.asarray(inputs["x"], np.float32)
    M_q = np.asarray(inputs["M_q"], np.float32)
    M_k = np.asarray(inputs["M_k"], np.float32)
    M_v = np.asarray(inputs["M_v"], np.float32)
    wp = _pack_weights(M_q, M_k, M_v).astype(np.float16)

    in_maps = []
    for b in range(B):
        xt = x[b].T  # (130, 4096) view
        xpk = np.empty((KC, N), np.float16)
        xpk[0:64] = xt[0:64]       # x1 rows
        xpk[64:128] = xt[65:129]   # x2 rows
        in_maps.append({"xp": xpk, "wp": wp})
    return in_maps


def _unpack_outputs(inputs, res):
    x = np.asarray(inputs["x"], np.float32)
    B_q = np.asarray(inputs["B_q"], np.float32)[:, 0, 0]
    B_k = np.asarray(inputs["B_k"], np.float32)[:, 0, 0]
    s_mid = x[:, :, 64]
    s_last = x[:, :, 129]

    # (B, 128, 32, 1024) partition-major -> token-major (B, N, 1024)
    oc = np.stack([np.asarray(res.results[b]["o"]) for b in range(B)])
    oc = oc.transpose(0, 2, 1, 3).reshape(B, N, OC)
    kc = oc[:, :, 0:256]
    vc = oc[:, :, 256:768]
    qc = oc[:, :, 768:1024]

    def qk_full(c, pair_bias, high_bias):
        f = np.zeros((B, N, H, E), np.float32)
        f[:, :, :P, 65:129] = c.reshape(B, N, P, 64)
        f[:, :, :P, 129] = pair_bias
        f[:, :, P:, 65] = high_bias
        return f.reshape(B, N, HE)

    q = qk_full(qc, s_last[..., None] * B_q[:P], s_last[..., None] * B_q[P:])
    k = qk_full(kc, s_last[..., None] * B_k[:P], s_mid[..., None] * B_k[P:])
    v_full = np.zeros((B, N, H, E), np.float32)
    v_full[:, :, :, 65:129] = vc.reshape(B, N, H, 64)
    return q, k, v_full.reshape(B, N, HE)


def _run(inputs, trace=False):
    if "nc" not in _CACHE:
        _CACHE["nc"] = _build()
    nc = _CACHE["nc"]
    in_maps = _prep_inputs(inputs)
    res = run_bass_kernel_spmd(nc, in_maps, core_ids=list(range(B)), trace=trace)
    return _unpack_outputs(inputs, res), res


def kernel(**inputs):
    outs, _ = _run(inputs, trace=False)
    return outs
